# revision 58
# baseline (speedup 1.0000x reference)
"""Trainium2 Bass kernel for nn_MultiHeadAttention (RoPE MHA, B=2 S=2048 E=1024 H=16).

Sharding: tensor-parallel over heads — 2 heads per core on 8 cores. Each core
computes its heads' q/k/v projections, RoPE, attention, and the partial output
projection (its rows of Wo); the host sums the 8 partials and adds bo.

v2 schedule (vs baseline): normalize chain shortened (reciprocal_approx_fast
from PSUM + one PE expander matmul broadcasts 1/Z for both heads, no DRAM
bounce); bias evictions on DVE (ACT does exp only); out-projection of block
i-1 interleaved into block i's kt loop so the PE never idles (HAM stays at
full clock); q/k projections share one 2-bank PSUM tile; yp partials in bf16.
"""

import os
import sys
from contextlib import ExitStack

import numpy as np

for _p in ("/opt/trn_rl_repo", "/opt/pypackages"):
    if _p not in sys.path and os.path.isdir(_p):
        sys.path.append(_p)

import concourse.bass as bass
import concourse.mybir as mybir
import concourse.tile as tile
from concourse import bacc
from concourse import bass_utils
from concourse.masks import make_identity

F32 = mybir.dt.float32
BF16 = mybir.dt.bfloat16
AF = mybir.ActivationFunctionType
OP = mybir.AluOpType

B = 2
S = 2048
E = 1024
H = 16
D = 64
N_CORES = 8
HPC = H // N_CORES  # heads per core = 2
HD = HPC * D  # 128

LAST_RESULTS = None  # BassKernelResults of the most recent run (for test harness)
_NC_CACHE = {}


def build_mha_nc():
    T = B * S
    TC = 512  # token chunk for projections
    NCH = T // TC  # 8
    QC = 512  # query chunk in attention
    NQC = S // QC  # 4
    NKT = S // 128  # key tiles per batch = 16
    KE = E // 128  # contraction tiles for projections = 8

    nc = bacc.Bacc(None, target_bir_lowering=False, debug=False)

    # xTt: pre-tiled x — chunk c's 8 contraction tiles contiguous:
    # xTt[p, (8c+k)*TC + j] = x[T=512c+j, e=128k+p]
    xTt = nc.dram_tensor("xTt", [128, (E // 128) * T], BF16, kind="ExternalInput")
    # wqkv: per k-tile [wq_k | wk_k | wv_k] packed: [128, 8*384]
    wqkv = nc.dram_tensor("wqkv", [128, (E // 128) * 3 * HD], BF16, kind="ExternalInput")
    bqkv = nc.dram_tensor("bqkv", [HD, 3], F32, kind="ExternalInput")
    wo = nc.dram_tensor("wo", [HD, E], BF16, kind="ExternalInput")
    # csT: per chunk [cos_c | sin_c] contiguous: [128, 2*T]
    csT = nc.dram_tensor("csT", [HD, 2 * T], BF16, kind="ExternalInput")
    rot = nc.dram_tensor("rot", [HD, HD], BF16, kind="ExternalInput")
    ones2 = nc.dram_tensor("ones2", [1, HD], BF16, kind="ExternalInput")
    yp = nc.dram_tensor("yp", [T, E], BF16, kind="ExternalOutput")
    DBG = os.environ.get("MHA_DEBUG") == "1"
    if DBG:
        dbg_zrec = nc.dram_tensor("dbg_zrec", [1, 1024], F32, kind="ExternalOutput")
        dbg_zb = nc.dram_tensor("dbg_zb", [128, 1024], F32, kind="ExternalOutput")
        dbg_ctx = nc.dram_tensor("dbg_ctx", [128, 512], F32, kind="ExternalOutput")

    scale = 1.0 / np.sqrt(D)

    with tile.TileContext(nc) as tc, ExitStack() as ctx:
        const = ctx.enter_context(tc.tile_pool(name="const", bufs=1))
        xt_pool = ctx.enter_context(tc.tile_pool(name="xt", bufs=2))
        cs_pool = ctx.enter_context(tc.tile_pool(name="cs", bufs=4))
        qkraw_pool = ctx.enter_context(tc.tile_pool(name="qkraw", bufs=4))
        rope_tmp = ctx.enter_context(tc.tile_pool(name="ropetmp", bufs=4))
        persist = ctx.enter_context(tc.tile_pool(name="persist", bufs=1))
        exps_pool = ctx.enter_context(tc.tile_pool(name="exps", bufs=10))
        zr_pool = ctx.enter_context(tc.tile_pool(name="zr", bufs=2))
        osb_pool = ctx.enter_context(tc.tile_pool(name="osb", bufs=6))

        # PSUM: A = 2 slots x 2 banks (pss / psqk), B = 2 slots x 1 bank
        # (psv/psrot/pvt/zb/pso), C = 2 slots x 1 bank (psc). Total 8 banks.
        ps_a = ctx.enter_context(tc.tile_pool(name="ps_a", bufs=2, space="PSUM"))
        ps_b = ctx.enter_context(tc.tile_pool(name="ps_b", bufs=2, space="PSUM"))
        ps_c = ctx.enter_context(tc.tile_pool(name="ps_c", bufs=2, space="PSUM"))

        # ---- constants to SBUF (gpsimd queue; off the sync DMA path) ----
        def load_const(name, dram_t, shape, dt):
            t = const.tile(shape, dt, name=name, tag=name)
            nc.gpsimd.dma_start(t[:], dram_t.ap())
            return t

        wqkv_a = const.tile([128, 2 * 3 * HD], BF16, name="wqkv_a", tag="wqkv_a")
        nc.gpsimd.dma_start(wqkv_a[:], wqkv.ap()[:, 0:768])
        wqkv_b = const.tile([128, (KE - 2) * 3 * HD], BF16, name="wqkv_b", tag="wqkv_b")
        nc.gpsimd.dma_start(wqkv_b[:], wqkv.ap()[:, 768:])

        # stationary slices: w (0=q,1=k,2=v), contraction tile k
        def w_sl(w, k):
            if k < 2:
                o = 384 * k + HD * w
                return wqkv_a[:, o : o + HD]
            o = 384 * (k - 2) + HD * w
            return wqkv_b[:, o : o + HD]

        bqkv_sb = load_const("bqkv_sb", bqkv, [HD, 3], F32)
        wo_sb = load_const("wo_sb", wo, [HD, E], BF16)
        rot_sb = load_const("rot_sb", rot, [HD, HD], BF16)
        ones2_sb = load_const("ones2_sb", ones2, [1, HD], BF16)
        ident = const.tile([128, 128], BF16, name="ident", tag="ident")
        make_identity(nc, ident)

        # ---- persistent intermediates ----
        q_rope = persist.tile([HD, T], BF16, name="q_rope", tag="q_rope")
        k_rope = persist.tile([HD, T], BF16, name="k_rope", tag="k_rope")
        v_sb = []
        for i in range(T // 128):
            t = persist.tile([128, HPC * (D + 1)], BF16, name=f"v_{i}", tag=f"v_{i}")
            for h in range(HPC):
                nc.vector.memset(t[:, (D + 1) * h + D : (D + 1) * (h + 1)], 1.0)
            v_sb.append(t)
        ctx_pack = {}
        for b in range(B):
            ctx_pack[b] = persist.tile([HD, S], BF16, name=f"ctxp_{b}", tag=f"ctxp_{b}")

        # ---- stage 1 chunk: projections (qT/kT/vT) + RoPE + v transpose ----
        def proj_chunk(c):
            c0 = TC * c
            xc0 = KE * TC * c
            # split into 4 pipelined loads: k-tiles are consumed in order, so
            # early tiles land fast and late tiles stream in behind compute
            xt = []
            for gi, (lo, hi) in enumerate(((0, 1), (1, 3), (3, 6), (6, 8))):
                g = xt_pool.tile(
                    [128, (hi - lo) * TC], BF16, name=f"xt_{c}_{gi}", tag=f"xt{gi}"
                )
                nc.sync.dma_start(
                    g[:], xTt.ap()[:, xc0 + lo * TC : xc0 + hi * TC]
                )
                xt += [g[:, TC * k : TC * (k + 1)] for k in range(hi - lo)]
            cs_c = cs_pool.tile([HD, 2 * TC], BF16, name="cs_c", tag="cs_c")
            nc.sync.dma_start(cs_c[:], csT.ap()[:, 2 * TC * c : 2 * TC * (c + 1)])
            cos_c = cs_c[:, 0:TC]
            sin_c = cs_c[:, TC : 2 * TC]

            # q and k projections share one 2-bank PSUM tile (q: cols 0-511,
            # k: cols 512-1023)
            psqk = ps_a.tile([HD, 2 * TC], F32, name="psqk", tag="ps_a")
            for half in (0, 1):
                for k in range(KE):
                    nc.tensor.matmul(
                        psqk[:, TC * half : TC * (half + 1)], w_sl(half, k), xt[k],
                        start=(k == 0), stop=(k == KE - 1),
                    )
            psv = ps_b.tile([HD, TC], F32, name="psv", tag="ps_b")
            for k in range(KE):
                nc.tensor.matmul(
                    psv[:], w_sl(2, k), xt[k],
                    start=(k == 0), stop=(k == KE - 1),
                )
            # q/k: bias evict on DVE, rotate matmul, rope combine DVE+GpSimd
            for half, out in ((0, q_rope), (1, k_rope)):
                raw = qkraw_pool.tile([HD, TC], BF16, name=f"raw{half}", tag="qkraw")
                nc.vector.tensor_scalar_add(
                    raw[:], psqk[:, TC * half : TC * (half + 1)],
                    bqkv_sb[:, half : half + 1],
                )
                psrot = ps_b.tile([HD, TC], F32, name="psrot", tag="ps_b")
                nc.tensor.matmul(psrot[:], rot_sb[:], raw[:], start=True, stop=True)
                sprod = rope_tmp.tile([HD, TC], F32, name="sprod", tag="ropetmp")
                nc.vector.tensor_tensor(sprod[:], psrot[:], sin_c[:], op=OP.mult)
                cprod = rope_tmp.tile([HD, TC], BF16, name="cprod", tag="ropetmp")
                nc.vector.tensor_tensor(cprod[:], raw[:], cos_c[:], op=OP.mult)
                nc.gpsimd.tensor_tensor(
                    out[:, c0 : c0 + TC], cprod[:], sprod[:], op=OP.add
                )
            # vT -> v natural via PE transpose; bias evict on DVE
            vraw = qkraw_pool.tile([HD, TC], BF16, name="vraw", tag="qkraw")
            nc.vector.tensor_scalar_add(vraw[:], psv[:], bqkv_sb[:, 2:3])
            for j in range(TC // 128):
                pvt = ps_b.tile([128, 128], BF16, name="pvt", tag="ps_b")
                nc.tensor.transpose(pvt[:], vraw[:, 128 * j : 128 * (j + 1)], ident[:])
                vt = v_sb[(c0 + 128 * j) // 128]
                for h in range(HPC):
                    nc.vector.tensor_copy(
                        vt[:, (D + 1) * h : (D + 1) * h + D],
                        pvt[:, D * h : D * (h + 1)],
                    )

        # ---- out-projection unit: one (j, e) tile of a finished block ----
        yp_engs = [nc.sync, nc.gpsimd]
        yp_rr = [0]

        def outproj_unit(b, j, e, tail=False):
            t0 = b * S
            pso = ps_b.tile([128, 512], F32, name="pso", tag="ps_b")
            nc.tensor.matmul(
                pso[:],
                ctx_pack[b][:, 128 * j : 128 * (j + 1)],
                wo_sb[:, 512 * e : 512 * (e + 1)],
                start=True, stop=True,
            )
            osb = osb_pool.tile([128, 512], BF16, name="osb", tag="osb")
            if tail and e == 1:
                # drain faster at the end: ACT is idle there, split evictions
                nc.scalar.activation(osb[:], pso[:], AF.Identity)
            else:
                nc.vector.tensor_copy(osb[:], pso[:])
            engs = yp_engs if not tail else [nc.sync, nc.gpsimd, nc.scalar]
            eng = engs[yp_rr[0] % len(engs)]
            yp_rr[0] += 1
            eng.dma_start(
                yp.ap()[t0 + 128 * j : t0 + 128 * (j + 1), 512 * e : 512 * (e + 1)],
                osb[:],
            )

        def outproj_units_of(b, qc):
            # 4 j-tiles x 2 e-halves for query chunk qc of batch b
            return [(b, j, e) for j in range(4 * qc, 4 * (qc + 1)) for e in range(2)]

        # ---- stage 2 block: attention for one (batch, query-chunk) ----
        # fill-task queue: PE/DVE work of finished blocks (normalize finish,
        # out-projection units) spliced into later blocks' kt loops so the PE
        # instruction stream never blocks on a slow serial chain.
        # deferred work spliced into later blocks' kt loops. norm2 of block
        # i pops at slot 3 of block i+1; out-proj units of block i pop in
        # block i+2 (two-block lag: every dependency is >=1 full block old,
        # so a popped PE instruction never stalls the in-order PE stream).
        sched = {"n2": None, "ops1": [], "ops2": []}

        def pop_fill(kt, last=False):
            if kt == 3 and sched["n2"] is not None:
                sched["n2"]()
                sched["n2"] = None
            elif kt >= 4 and sched["ops2"]:
                outproj_unit(*sched["ops2"].pop(0))
                # last block: drain the (two-block-old, dependency-settled)
                # backlog harder so the tail shrinks
                if last and sched["ops2"]:
                    outproj_unit(*sched["ops2"].pop(0))

        def qc_block(b, qc, first, last=False):
            t0 = b * S
            q0 = t0 + QC * qc
            psc = [
                ps_c.tile([D + 1, QC], F32, name=f"psctx{h}", tag="ps_c")
                for h in range(HPC)
            ]
            ex_stash = {}
            # defer the first ctx writes: the previous block's psc eviction
            # (in its norm1, at the end of the previous kt loop) must land
            kt_start = 0 if first else 2

            def ctx_mms(kt, ex, start):
                k0 = t0 + 128 * kt
                for h in range(HPC):
                    nc.tensor.matmul(
                        psc[h][:],
                        v_sb[k0 // 128][:, (D + 1) * h : (D + 1) * (h + 1)],
                        ex[:, QC * h : QC * (h + 1)],
                        start=start, stop=(kt == NKT - 1),
                    )

            for kt in range(NKT):
                k0 = t0 + 128 * kt
                # both heads' scores into one 2-bank psum tile -> single exp
                pss = ps_a.tile([128, HPC * QC], F32, name="pss", tag="ps_a")
                for h in range(HPC):
                    nc.tensor.matmul(
                        pss[:, QC * h : QC * (h + 1)],
                        k_rope[D * h : D * (h + 1), k0 : k0 + 128],
                        q_rope[D * h : D * (h + 1), q0 : q0 + QC],
                        start=True, stop=True,
                    )
                ex = exps_pool.tile([128, HPC * QC], BF16, name="ex", tag="exps")
                nc.scalar.activation(ex[:], pss[:], AF.Exp, scale=scale)
                pop_fill(kt, last)
                if kt < kt_start:
                    ex_stash[kt] = ex
                else:
                    ctx_mms(kt, ex, start=(kt == kt_start))
                    if kt - kt_start in ex_stash:
                        ctx_mms(kt - kt_start, ex_stash.pop(kt - kt_start), False)

            # normalize chain (DVE only, emitted inline): Z -> 1/Z -> bf16,
            # then evict ctx rows (frees psc for the next block)
            zsb = zr_pool.tile([1, HPC * QC], F32, name="zsb", tag="zsb")
            for h in range(HPC):
                nc.vector.tensor_copy(
                    zsb[0:1, QC * h : QC * (h + 1)], psc[h][D : D + 1, :]
                )
            zrec = zr_pool.tile([1, HPC * QC], F32, name="zrec", tag="zr")
            nc.vector.reciprocal_approx_fast(zrec[:], zsb[:])
            zrb = zr_pool.tile([1, HPC * QC], BF16, name="zrb", tag="zrb")
            nc.vector.tensor_copy(zrb[:], zrec[:])
            cun = zr_pool.tile([128, QC], BF16, name="cun", tag="cun")
            for h in range(HPC):
                nc.vector.tensor_copy(cun[D * h : D * (h + 1), :], psc[h][0:D, :])

            # expander matmuls + normalize multiply; popped at kt3 next block
            def norm2():
                for h in range(HPC):
                    zbh = ps_b.tile([D, QC], F32, name=f"zb{h}", tag="ps_b")
                    nc.tensor.matmul(
                        zbh[:], ones2_sb[0:1, 0:D],
                        zrb[0:1, QC * h : QC * (h + 1)],
                        start=True, stop=True,
                    )
                    nc.vector.tensor_tensor(
                        ctx_pack[b][D * h : D * (h + 1), QC * qc : QC * (qc + 1)],
                        cun[D * h : D * (h + 1), :], zbh[:], op=OP.mult,
                    )

            sched["n2"] = norm2
            sched["ops2"] = sched["ops2"] + sched["ops1"]
            sched["ops1"] = outproj_units_of(b, qc)

        # ---- interleaved emission ----
        # chunks 0-3 (batch 0); then blocks of batch 0 with chunks 4-7
        # interleaved; blocks of batch 1; out-proj lags one block, spliced
        # into the next block's kt loop.
        NCB = NCH // B  # projection chunks per batch = 4
        for c in range(NCB):
            proj_chunk(c)
        blocks = [(b, qc) for b in range(B) for qc in range(NQC)]
        for idx, (b, qc) in enumerate(blocks):
            if idx < NCB:
                proj_chunk(NCB + idx)
            qc_block(b, qc, first=(idx == 0), last=(idx == len(blocks) - 1))
        # tail drain: the last block's normalize and the last two blocks'
        # out-proj units (evictions split across ACT and DVE)
        sched["n2"]()
        for u in sched["ops2"] + sched["ops1"]:
            outproj_unit(*u, tail=True)

    nc.compile()
    return nc


def _rope_tables():
    inv_freq = 1.0 / (10000.0 ** (np.arange(0, D, 2, dtype=np.float32) / D))
    t = np.arange(S, dtype=np.float32)
    freqs = np.outer(t, inv_freq).astype(np.float32)
    emb = np.concatenate([freqs, freqs], axis=-1)
    return np.cos(emb).astype(np.float32), np.sin(emb).astype(np.float32)


def _rot_matrix():
    R = np.zeros((HD, HD), np.float32)
    for hh in range(HPC):
        for do in range(D):
            po = D * hh + do
            if do < D // 2:
                R[D * hh + do + D // 2, po] = -1.0
            else:
                R[D * hh + do - D // 2, po] = 1.0
    return R


def kernel(x, Wq, bq, Wk, bk, Wv, bv, Wo, bo):
    global LAST_RESULTS
    import ml_dtypes

    x = np.asarray(x, dtype=np.float32)
    Wq, bq = np.asarray(Wq, np.float32), np.asarray(bq, np.float32)
    Wk, bk = np.asarray(Wk, np.float32), np.asarray(bk, np.float32)
    Wv, bv = np.asarray(Wv, np.float32), np.asarray(bv, np.float32)
    Wo, bo = np.asarray(Wo, np.float32), np.asarray(bo, np.float32)

    dt_np = ml_dtypes.bfloat16
    T = B * S

    if "nc" not in _NC_CACHE:
        _NC_CACHE["nc"] = build_mha_nc()
    nc = _NC_CACHE["nc"]

    TC = 512
    KE = E // 128
    # xTt: [128, KE*T] — chunk c's 8 contraction tiles contiguous
    # xTt[p, (8c+k)*TC + j] = x[t=512c+j, e=128k+p]
    xf = x.reshape(T, E)  # [t, e]
    xTt = (
        xf.reshape(T // TC, TC, KE, 128)  # [c, j, k, p]
        .transpose(3, 0, 2, 1)  # [p, c, k, j]
        .reshape(128, KE * T)
    )
    xTt = np.ascontiguousarray(xTt).astype(dt_np)
    cos, sin = _rope_tables()
    cosT = np.tile(np.ascontiguousarray(cos.T), (HPC, B))  # [HD, T]
    sinT = np.tile(np.ascontiguousarray(sin.T), (HPC, B))
    # csT: per chunk [cos_c | sin_c]: [HD, 2T]
    csT = np.concatenate(
        [
            np.concatenate(
                [cosT[:, TC * c : TC * (c + 1)], sinT[:, TC * c : TC * (c + 1)]],
                axis=1,
            )
            for c in range(T // TC)
        ],
        axis=1,
    ).astype(dt_np)
    R = _rot_matrix().astype(dt_np)
    ones2 = np.ones((1, HD), np.float32).astype(dt_np)

    in_maps = []
    for c in range(N_CORES):
        sl = slice(HD * c, HD * (c + 1))
        # wqkv: per k-tile [wq_k | wk_k | wv_k]: [128, KE*384]
        wqs, wks, wvs = Wq[:, sl], Wk[:, sl], Wv[:, sl]
        wqkv = np.concatenate(
            [
                np.concatenate(
                    [w[128 * k : 128 * (k + 1), :] for w in (wqs, wks, wvs)], axis=1
                )
                for k in range(KE)
            ],
            axis=1,
        ).astype(dt_np)
        bqkv = np.stack([bq[sl], bk[sl], bv[sl]], axis=1).astype(np.float32)
        in_maps.append(
            {
                "xTt": xTt,
                "wqkv": np.ascontiguousarray(wqkv),
                "bqkv": np.ascontiguousarray(bqkv),
                "wo": np.ascontiguousarray(Wo[sl, :]).astype(dt_np),
                "csT": csT,
                "rot": R,
                "ones2": ones2,
            }
        )

    res = bass_utils.run_bass_kernel_spmd(nc, in_maps, core_ids=list(range(N_CORES)))
    LAST_RESULTS = res

    out = np.zeros((T, E), np.float64)
    for c in range(N_CORES):
        out += res.results[c]["yp"].astype(np.float64)
    out += bo.astype(np.float64)
    return out.astype(np.float32).reshape(B, S, E)


# revision 59
# speedup vs baseline: 1.0095x; 1.0095x over previous
"""Trainium2 Bass kernel for nn_MultiHeadAttention (RoPE MHA, B=2 S=2048 E=1024 H=16).

Sharding: tensor-parallel over heads — 2 heads per core on 8 cores. Each core
computes its heads' q/k/v projections, RoPE, attention, and the partial output
projection (its rows of Wo); the host sums the 8 partials and adds bo.

v2 schedule (vs baseline): normalize chain shortened (reciprocal_approx_fast
from PSUM + one PE expander matmul broadcasts 1/Z for both heads, no DRAM
bounce); bias evictions on DVE (ACT does exp only); out-projection of block
i-1 interleaved into block i's kt loop so the PE never idles (HAM stays at
full clock); q/k projections share one 2-bank PSUM tile; yp partials in bf16.
"""

import os
import sys
from contextlib import ExitStack

import numpy as np

for _p in ("/opt/trn_rl_repo", "/opt/pypackages"):
    if _p not in sys.path and os.path.isdir(_p):
        sys.path.append(_p)

import concourse.bass as bass
import concourse.mybir as mybir
import concourse.tile as tile
from concourse import bacc
from concourse import bass_utils
from concourse.masks import make_identity

F32 = mybir.dt.float32
BF16 = mybir.dt.bfloat16
AF = mybir.ActivationFunctionType
OP = mybir.AluOpType

B = 2
S = 2048
E = 1024
H = 16
D = 64
N_CORES = 8
HPC = H // N_CORES  # heads per core = 2
HD = HPC * D  # 128

LAST_RESULTS = None  # BassKernelResults of the most recent run (for test harness)
_NC_CACHE = {}


def build_mha_nc():
    T = B * S
    TC = 512  # token chunk for projections
    NCH = T // TC  # 8
    QC = 512  # query chunk in attention
    NQC = S // QC  # 4
    NKT = S // 128  # key tiles per batch = 16
    KE = E // 128  # contraction tiles for projections = 8

    nc = bacc.Bacc(None, target_bir_lowering=False, debug=False)

    # xTt: pre-tiled x — chunk c's 8 contraction tiles contiguous:
    # xTt[p, (8c+k)*TC + j] = x[T=512c+j, e=128k+p]
    xTt = nc.dram_tensor("xTt", [128, (E // 128) * T], BF16, kind="ExternalInput")
    # wqkv: per k-tile [wq_k | wk_k | wv_k] packed: [128, 8*384]
    wqkv = nc.dram_tensor("wqkv", [128, (E // 128) * 3 * HD], BF16, kind="ExternalInput")
    bqkv = nc.dram_tensor("bqkv", [HD, 3], F32, kind="ExternalInput")
    wo = nc.dram_tensor("wo", [HD, E], BF16, kind="ExternalInput")
    # csT: per chunk [cos_c | sin_c] contiguous: [128, 2*T]
    csT = nc.dram_tensor("csT", [HD, 2 * T], BF16, kind="ExternalInput")
    rot = nc.dram_tensor("rot", [HD, HD], BF16, kind="ExternalInput")
    ones2 = nc.dram_tensor("ones2", [1, HD], BF16, kind="ExternalInput")
    yp = nc.dram_tensor("yp", [T, E], BF16, kind="ExternalOutput")
    DBG = os.environ.get("MHA_DEBUG") == "1"
    if DBG:
        dbg_zrec = nc.dram_tensor("dbg_zrec", [1, 1024], F32, kind="ExternalOutput")
        dbg_zb = nc.dram_tensor("dbg_zb", [128, 1024], F32, kind="ExternalOutput")
        dbg_ctx = nc.dram_tensor("dbg_ctx", [128, 512], F32, kind="ExternalOutput")

    scale = 1.0 / np.sqrt(D)

    with tile.TileContext(nc) as tc, ExitStack() as ctx:
        const = ctx.enter_context(tc.tile_pool(name="const", bufs=1))
        xt_pool = ctx.enter_context(tc.tile_pool(name="xt", bufs=2))
        cs_pool = ctx.enter_context(tc.tile_pool(name="cs", bufs=4))
        qkraw_pool = ctx.enter_context(tc.tile_pool(name="qkraw", bufs=4))
        rope_tmp = ctx.enter_context(tc.tile_pool(name="ropetmp", bufs=4))
        persist = ctx.enter_context(tc.tile_pool(name="persist", bufs=1))
        exps_pool = ctx.enter_context(tc.tile_pool(name="exps", bufs=10))
        zr_pool = ctx.enter_context(tc.tile_pool(name="zr", bufs=2))
        osb_pool = ctx.enter_context(tc.tile_pool(name="osb", bufs=6))

        # PSUM: A = 2 slots x 2 banks (pss / psqk), B = 2 slots x 1 bank
        # (psv/psrot/pvt/zb/pso), C = 2 slots x 1 bank (psc). Total 8 banks.
        ps_a = ctx.enter_context(tc.tile_pool(name="ps_a", bufs=2, space="PSUM"))
        ps_b = ctx.enter_context(tc.tile_pool(name="ps_b", bufs=2, space="PSUM"))
        ps_c = ctx.enter_context(tc.tile_pool(name="ps_c", bufs=2, space="PSUM"))

        # ---- constants to SBUF (gpsimd queue; off the sync DMA path) ----
        def load_const(name, dram_t, shape, dt):
            t = const.tile(shape, dt, name=name, tag=name)
            nc.gpsimd.dma_start(t[:], dram_t.ap())
            return t

        wqkv_a = const.tile([128, 2 * 3 * HD], BF16, name="wqkv_a", tag="wqkv_a")
        nc.gpsimd.dma_start(wqkv_a[:], wqkv.ap()[:, 0:768])
        wqkv_b = const.tile([128, (KE - 2) * 3 * HD], BF16, name="wqkv_b", tag="wqkv_b")
        nc.gpsimd.dma_start(wqkv_b[:], wqkv.ap()[:, 768:])

        # stationary slices: w (0=q,1=k,2=v), contraction tile k
        def w_sl(w, k):
            if k < 2:
                o = 384 * k + HD * w
                return wqkv_a[:, o : o + HD]
            o = 384 * (k - 2) + HD * w
            return wqkv_b[:, o : o + HD]

        bqkv_sb = load_const("bqkv_sb", bqkv, [HD, 3], F32)
        wo_sb = load_const("wo_sb", wo, [HD, E], BF16)
        rot_sb = load_const("rot_sb", rot, [HD, HD], BF16)
        ones2_sb = load_const("ones2_sb", ones2, [1, HD], BF16)
        ident = const.tile([128, 128], BF16, name="ident", tag="ident")
        make_identity(nc, ident)

        # ---- persistent intermediates ----
        q_rope = persist.tile([HD, T], BF16, name="q_rope", tag="q_rope")
        k_rope = persist.tile([HD, T], BF16, name="k_rope", tag="k_rope")
        v_sb = []
        for i in range(T // 128):
            t = persist.tile([128, HPC * (D + 1)], BF16, name=f"v_{i}", tag=f"v_{i}")
            for h in range(HPC):
                nc.vector.memset(t[:, (D + 1) * h + D : (D + 1) * (h + 1)], 1.0)
            v_sb.append(t)
        ctx_pack = {}
        for b in range(B):
            ctx_pack[b] = persist.tile([HD, S], BF16, name=f"ctxp_{b}", tag=f"ctxp_{b}")

        # ---- stage 1 chunk: projections (qT/kT/vT) + RoPE + v transpose ----
        def proj_chunk(c):
            c0 = TC * c
            xc0 = KE * TC * c
            # split into pipelined loads: k-tiles are consumed in order, so
            # early tiles land fast and late tiles stream in behind compute
            xt = []
            for gi, (lo, hi) in enumerate(((0, 1), (1, 8))):
                g = xt_pool.tile(
                    [128, (hi - lo) * TC], BF16, name=f"xt_{c}_{gi}", tag=f"xt{gi}"
                )
                nc.sync.dma_start(
                    g[:], xTt.ap()[:, xc0 + lo * TC : xc0 + hi * TC]
                )
                xt += [g[:, TC * k : TC * (k + 1)] for k in range(hi - lo)]
            cs_c = cs_pool.tile([HD, 2 * TC], BF16, name="cs_c", tag="cs_c")
            nc.sync.dma_start(cs_c[:], csT.ap()[:, 2 * TC * c : 2 * TC * (c + 1)])
            cos_c = cs_c[:, 0:TC]
            sin_c = cs_c[:, TC : 2 * TC]

            # q and k projections share one 2-bank PSUM tile (q: cols 0-511,
            # k: cols 512-1023)
            psqk = ps_a.tile([HD, 2 * TC], F32, name="psqk", tag="ps_a")
            for half in (0, 1):
                for k in range(KE):
                    nc.tensor.matmul(
                        psqk[:, TC * half : TC * (half + 1)], w_sl(half, k), xt[k],
                        start=(k == 0), stop=(k == KE - 1),
                    )
            psv = ps_b.tile([HD, TC], F32, name="psv", tag="ps_b")
            for k in range(KE):
                nc.tensor.matmul(
                    psv[:], w_sl(2, k), xt[k],
                    start=(k == 0), stop=(k == KE - 1),
                )
            # q/k: bias evict on DVE, rotate matmul, rope combine DVE+GpSimd
            for half, out in ((0, q_rope), (1, k_rope)):
                raw = qkraw_pool.tile([HD, TC], BF16, name=f"raw{half}", tag="qkraw")
                nc.vector.tensor_scalar_add(
                    raw[:], psqk[:, TC * half : TC * (half + 1)],
                    bqkv_sb[:, half : half + 1],
                )
                psrot = ps_b.tile([HD, TC], F32, name="psrot", tag="ps_b")
                nc.tensor.matmul(psrot[:], rot_sb[:], raw[:], start=True, stop=True)
                sprod = rope_tmp.tile([HD, TC], F32, name="sprod", tag="ropetmp")
                nc.vector.tensor_tensor(sprod[:], psrot[:], sin_c[:], op=OP.mult)
                cprod = rope_tmp.tile([HD, TC], BF16, name="cprod", tag="ropetmp")
                nc.vector.tensor_tensor(cprod[:], raw[:], cos_c[:], op=OP.mult)
                nc.gpsimd.tensor_tensor(
                    out[:, c0 : c0 + TC], cprod[:], sprod[:], op=OP.add
                )
            # vT -> v natural via PE transpose; bias evict on DVE
            vraw = qkraw_pool.tile([HD, TC], BF16, name="vraw", tag="qkraw")
            nc.vector.tensor_scalar_add(vraw[:], psv[:], bqkv_sb[:, 2:3])
            for j in range(TC // 128):
                pvt = ps_b.tile([128, 128], BF16, name="pvt", tag="ps_b")
                nc.tensor.transpose(pvt[:], vraw[:, 128 * j : 128 * (j + 1)], ident[:])
                vt = v_sb[(c0 + 128 * j) // 128]
                for h in range(HPC):
                    nc.vector.tensor_copy(
                        vt[:, (D + 1) * h : (D + 1) * h + D],
                        pvt[:, D * h : D * (h + 1)],
                    )

        # ---- out-projection unit: one (j, e) tile of a finished block ----
        yp_engs = [nc.sync, nc.gpsimd]
        yp_rr = [0]

        def outproj_unit(b, j, e, tail=False):
            t0 = b * S
            pso = ps_b.tile([128, 512], F32, name="pso", tag="ps_b")
            nc.tensor.matmul(
                pso[:],
                ctx_pack[b][:, 128 * j : 128 * (j + 1)],
                wo_sb[:, 512 * e : 512 * (e + 1)],
                start=True, stop=True,
            )
            osb = osb_pool.tile([128, 512], BF16, name="osb", tag="osb")
            if tail and e == 1:
                # drain faster at the end: ACT is idle there, split evictions
                nc.scalar.activation(osb[:], pso[:], AF.Identity)
            else:
                nc.vector.tensor_copy(osb[:], pso[:])
            engs = yp_engs if not tail else [nc.sync, nc.gpsimd, nc.scalar]
            eng = engs[yp_rr[0] % len(engs)]
            yp_rr[0] += 1
            eng.dma_start(
                yp.ap()[t0 + 128 * j : t0 + 128 * (j + 1), 512 * e : 512 * (e + 1)],
                osb[:],
            )

        def outproj_units_of(b, qc):
            # 4 j-tiles x 2 e-halves for query chunk qc of batch b
            return [(b, j, e) for j in range(4 * qc, 4 * (qc + 1)) for e in range(2)]

        # ---- stage 2 block: attention for one (batch, query-chunk) ----
        # fill-task queue: PE/DVE work of finished blocks (normalize finish,
        # out-projection units) spliced into later blocks' kt loops so the PE
        # instruction stream never blocks on a slow serial chain.
        # deferred work spliced into later blocks' kt loops. norm2 of block
        # i pops at slot 3 of block i+1; out-proj units of block i pop in
        # block i+2 (two-block lag: every dependency is >=1 full block old,
        # so a popped PE instruction never stalls the in-order PE stream).
        sched = {"n2": None, "ops1": [], "ops2": []}

        def pop_fill(kt, last=False):
            if kt == 3 and sched["n2"] is not None:
                sched["n2"]()
                sched["n2"] = None
            elif kt >= 4 and sched["ops2"]:
                outproj_unit(*sched["ops2"].pop(0))
                # last block: drain the (two-block-old, dependency-settled)
                # backlog harder so the tail shrinks
                if last and sched["ops2"]:
                    outproj_unit(*sched["ops2"].pop(0))

        def qc_block(b, qc, first, last=False):
            t0 = b * S
            q0 = t0 + QC * qc
            psc = [
                ps_c.tile([D + 1, QC], F32, name=f"psctx{h}", tag="ps_c")
                for h in range(HPC)
            ]
            ex_stash = {}
            # defer the first ctx writes: the previous block's psc eviction
            # (in its norm1, at the end of the previous kt loop) must land
            kt_start = 0 if first else 2

            def ctx_mms(kt, ex, start):
                k0 = t0 + 128 * kt
                for h in range(HPC):
                    nc.tensor.matmul(
                        psc[h][:],
                        v_sb[k0 // 128][:, (D + 1) * h : (D + 1) * (h + 1)],
                        ex[:, QC * h : QC * (h + 1)],
                        start=start, stop=(kt == NKT - 1),
                    )

            for kt in range(NKT):
                k0 = t0 + 128 * kt
                # both heads' scores into one 2-bank psum tile -> single exp
                pss = ps_a.tile([128, HPC * QC], F32, name="pss", tag="ps_a")
                for h in range(HPC):
                    nc.tensor.matmul(
                        pss[:, QC * h : QC * (h + 1)],
                        k_rope[D * h : D * (h + 1), k0 : k0 + 128],
                        q_rope[D * h : D * (h + 1), q0 : q0 + QC],
                        start=True, stop=True,
                    )
                ex = exps_pool.tile([128, HPC * QC], BF16, name="ex", tag="exps")
                nc.scalar.activation(ex[:], pss[:], AF.Exp, scale=scale)
                pop_fill(kt, last)
                if kt < kt_start:
                    ex_stash[kt] = ex
                else:
                    ctx_mms(kt, ex, start=(kt == kt_start))
                    if kt - kt_start in ex_stash:
                        ctx_mms(kt - kt_start, ex_stash.pop(kt - kt_start), False)

            # normalize chain (DVE only, emitted inline): Z -> 1/Z -> bf16,
            # then evict ctx rows (frees psc for the next block)
            zsb = zr_pool.tile([1, HPC * QC], F32, name="zsb", tag="zsb")
            for h in range(HPC):
                nc.vector.tensor_copy(
                    zsb[0:1, QC * h : QC * (h + 1)], psc[h][D : D + 1, :]
                )
            zrec = zr_pool.tile([1, HPC * QC], F32, name="zrec", tag="zr")
            nc.vector.reciprocal_approx_fast(zrec[:], zsb[:])
            zrb = zr_pool.tile([1, HPC * QC], BF16, name="zrb", tag="zrb")
            nc.vector.tensor_copy(zrb[:], zrec[:])
            cun = zr_pool.tile([128, QC], BF16, name="cun", tag="cun")
            for h in range(HPC):
                nc.vector.tensor_copy(cun[D * h : D * (h + 1), :], psc[h][0:D, :])

            # expander matmuls + normalize multiply; popped at kt3 next block
            def norm2():
                for h in range(HPC):
                    zbh = ps_b.tile([D, QC], F32, name=f"zb{h}", tag="ps_b")
                    nc.tensor.matmul(
                        zbh[:], ones2_sb[0:1, 0:D],
                        zrb[0:1, QC * h : QC * (h + 1)],
                        start=True, stop=True,
                    )
                    nc.vector.tensor_tensor(
                        ctx_pack[b][D * h : D * (h + 1), QC * qc : QC * (qc + 1)],
                        cun[D * h : D * (h + 1), :], zbh[:], op=OP.mult,
                    )

            sched["n2"] = norm2
            sched["ops2"] = sched["ops2"] + sched["ops1"]
            sched["ops1"] = outproj_units_of(b, qc)

        # ---- interleaved emission ----
        # chunks 0-3 (batch 0); then blocks of batch 0 with chunks 4-7
        # interleaved; blocks of batch 1; out-proj lags one block, spliced
        # into the next block's kt loop.
        NCB = NCH // B  # projection chunks per batch = 4
        for c in range(NCB):
            proj_chunk(c)
        blocks = [(b, qc) for b in range(B) for qc in range(NQC)]
        for idx, (b, qc) in enumerate(blocks):
            if idx < NCB:
                proj_chunk(NCB + idx)
            qc_block(b, qc, first=(idx == 0), last=(idx == len(blocks) - 1))
        # tail drain: the last block's normalize and the last two blocks'
        # out-proj units (evictions split across ACT and DVE)
        sched["n2"]()
        for u in sched["ops2"] + sched["ops1"]:
            outproj_unit(*u, tail=True)

    nc.compile()
    return nc


def _rope_tables():
    inv_freq = 1.0 / (10000.0 ** (np.arange(0, D, 2, dtype=np.float32) / D))
    t = np.arange(S, dtype=np.float32)
    freqs = np.outer(t, inv_freq).astype(np.float32)
    emb = np.concatenate([freqs, freqs], axis=-1)
    return np.cos(emb).astype(np.float32), np.sin(emb).astype(np.float32)


def _rot_matrix():
    R = np.zeros((HD, HD), np.float32)
    for hh in range(HPC):
        for do in range(D):
            po = D * hh + do
            if do < D // 2:
                R[D * hh + do + D // 2, po] = -1.0
            else:
                R[D * hh + do - D // 2, po] = 1.0
    return R


def kernel(x, Wq, bq, Wk, bk, Wv, bv, Wo, bo):
    global LAST_RESULTS
    import ml_dtypes

    x = np.asarray(x, dtype=np.float32)
    Wq, bq = np.asarray(Wq, np.float32), np.asarray(bq, np.float32)
    Wk, bk = np.asarray(Wk, np.float32), np.asarray(bk, np.float32)
    Wv, bv = np.asarray(Wv, np.float32), np.asarray(bv, np.float32)
    Wo, bo = np.asarray(Wo, np.float32), np.asarray(bo, np.float32)

    dt_np = ml_dtypes.bfloat16
    T = B * S

    if "nc" not in _NC_CACHE:
        _NC_CACHE["nc"] = build_mha_nc()
    nc = _NC_CACHE["nc"]

    TC = 512
    KE = E // 128
    # xTt: [128, KE*T] — chunk c's 8 contraction tiles contiguous
    # xTt[p, (8c+k)*TC + j] = x[t=512c+j, e=128k+p]
    xf = x.reshape(T, E)  # [t, e]
    xTt = (
        xf.reshape(T // TC, TC, KE, 128)  # [c, j, k, p]
        .transpose(3, 0, 2, 1)  # [p, c, k, j]
        .reshape(128, KE * T)
    )
    xTt = np.ascontiguousarray(xTt).astype(dt_np)
    cos, sin = _rope_tables()
    cosT = np.tile(np.ascontiguousarray(cos.T), (HPC, B))  # [HD, T]
    sinT = np.tile(np.ascontiguousarray(sin.T), (HPC, B))
    # csT: per chunk [cos_c | sin_c]: [HD, 2T]
    csT = np.concatenate(
        [
            np.concatenate(
                [cosT[:, TC * c : TC * (c + 1)], sinT[:, TC * c : TC * (c + 1)]],
                axis=1,
            )
            for c in range(T // TC)
        ],
        axis=1,
    ).astype(dt_np)
    R = _rot_matrix().astype(dt_np)
    ones2 = np.ones((1, HD), np.float32).astype(dt_np)

    in_maps = []
    for c in range(N_CORES):
        sl = slice(HD * c, HD * (c + 1))
        # wqkv: per k-tile [wq_k | wk_k | wv_k]: [128, KE*384]
        wqs, wks, wvs = Wq[:, sl], Wk[:, sl], Wv[:, sl]
        wqkv = np.concatenate(
            [
                np.concatenate(
                    [w[128 * k : 128 * (k + 1), :] for w in (wqs, wks, wvs)], axis=1
                )
                for k in range(KE)
            ],
            axis=1,
        ).astype(dt_np)
        bqkv = np.stack([bq[sl], bk[sl], bv[sl]], axis=1).astype(np.float32)
        in_maps.append(
            {
                "xTt": xTt,
                "wqkv": np.ascontiguousarray(wqkv),
                "bqkv": np.ascontiguousarray(bqkv),
                "wo": np.ascontiguousarray(Wo[sl, :]).astype(dt_np),
                "csT": csT,
                "rot": R,
                "ones2": ones2,
            }
        )

    res = bass_utils.run_bass_kernel_spmd(nc, in_maps, core_ids=list(range(N_CORES)))
    LAST_RESULTS = res

    out = np.zeros((T, E), np.float64)
    for c in range(N_CORES):
        out += res.results[c]["yp"].astype(np.float64)
    out += bo.astype(np.float64)
    return out.astype(np.float32).reshape(B, S, E)
